# revision 2
# baseline (speedup 1.0000x reference)
"""Distributed multi-head causal attention with RoPE on 8 TRN2 NeuronCores.

Sharding: batch (2) x head-groups (4 heads each) -> 8 cores.
  core c: batch b = c // 4, head group g = c % 4 (global heads 4g..4g+3).

Per-core kernel (all matmuls bf16, fp32 accumulate):
  1. QKV projections in transposed layout: QT/KT[d, seq] (head dim on
     partitions), V[seq, dv] natural.  RoPE pair-interleave is folded into a
     host-side permutation of wq/wk rows (evens-first), so on-device RoPE is
     3 elementwise ops + a half-swap copy.  The 1/sqrt(hd) scale is folded
     into wq on the host.
  2. Scores computed transposed: ST[k, q] = KT_blk.T @ QT (so softmax'd
     probabilities come out in the exact layout PV needs as its moving
     operand).  exp on ACT (no max subtraction -- scores are O(1) for this
     problem), causal blocks skipped structurally, partial blocks masked by
     a 0/1 pattern multiply.  Row sums via a ones-vector matmul; the
     normalization is applied to the PV output (per-column reciprocal
     broadcast across partitions).
  3. Per-head AllGather (bf16) of normalized attnT across the 4 cores of the
     batch group, overlapped with attention of later heads.
  4. Output projection column-sharded: each core computes its 512 output
     columns from the gathered attnT; host concatenates.
"""

import functools
import math

import numpy as np
import ml_dtypes

BSZ, SEQ, DIM, NH, HD = 2, 2048, 2048, 16, 128
NCORES = 8
GSIZE = 4            # cores per batch group
HPC = NH // GSIZE    # heads per core = 4
DLOC = HPC * HD      # local head dims = 512
QC = 512             # q-chunk (matmul moving free dim)
NQC = SEQ // QC      # 4
KT = 128             # k-tile
NKT = SEQ // KT      # 16
IC = 128             # contraction tile
NIC = DIM // IC      # 16
BF16 = ml_dtypes.bfloat16
NEG_BIG = -30000.0


def _build_and_compile(block_plan_key, n_pat):
    """Build + compile the SPMD bass graph.  block_plan_key is a tuple over
    q-chunks of tuples of (kt, pat_idx_or_-1)."""
    import concourse.bass as bass
    import concourse.tile as tile
    from concourse import bacc, mybir
    from contextlib import ExitStack

    f32 = mybir.dt.float32
    bf16 = mybir.dt.bfloat16
    ts = bass.ts

    block_plan = [[(kt, (None if p < 0 else p)) for kt, p in qcp]
                  for qcp in block_plan_key]

    nc = bacc.Bacc("TRN2", target_bir_lowering=False, debug=False,
                   num_devices=NCORES)

    xT_d = nc.dram_tensor("xT", [DIM, SEQ], bf16, kind="ExternalInput").ap()
    wqT_d = nc.dram_tensor("wqT", [DIM, DLOC], bf16, kind="ExternalInput").ap()
    wkT_d = nc.dram_tensor("wkT", [DIM, DLOC], bf16, kind="ExternalInput").ap()
    wvT_d = nc.dram_tensor("wvT", [DIM, DLOC], bf16, kind="ExternalInput").ap()
    woT_d = nc.dram_tensor("woT", [DIM, DLOC], bf16, kind="ExternalInput").ap()
    cos2_d = nc.dram_tensor("cos2", [HD, SEQ], bf16, kind="ExternalInput").ap()
    sinpm_d = nc.dram_tensor("sinpm", [HD, SEQ], bf16, kind="ExternalInput").ap()
    pat_d = nc.dram_tensor("pat", [max(n_pat, 1), KT, QC], bf16,
                           kind="ExternalInput").ap()
    out_d = nc.dram_tensor("out", [SEQ, DLOC], f32, kind="ExternalOutput").ap()

    groups = [[0, 1, 2, 3], [4, 5, 6, 7]]

    with tile.TileContext(nc) as tc, ExitStack() as top:
        persist = top.enter_context(tc.tile_pool(name="persist", bufs=1))
        dram = top.enter_context(
            tc.tile_pool(name="dram", bufs=2 * HPC, space="DRAM"))

        qt_sb = persist.tile([128, HPC, SEQ], bf16, name="qt_sb")
        kt_sb = persist.tile([128, HPC, SEQ], bf16, name="kt_sb")
        v_sb = persist.tile([128, NKT, DLOC], bf16, name="v_sb")
        at_sb = persist.tile([128, HPC, SEQ], bf16, name="at_sb")
        ones_sb = persist.tile([128, 1], bf16, name="ones_sb")
        pat_sb = persist.tile([128, max(n_pat, 1), QC], bf16, name="pat_sb")

        nc.vector.memset(ones_sb[:], 1.0)
        nc.sync.dma_start(out=pat_sb[:], in_=pat_d.rearrange("n p q -> p n q"))

        # ---------------- Phase A: QKV projections + RoPE ----------------
        with ExitStack() as pa:
            wpool = pa.enter_context(tc.tile_pool(name="wpool", bufs=1))
            xpool = pa.enter_context(tc.tile_pool(name="xpool", bufs=2))
            rpool = pa.enter_context(tc.tile_pool(name="rope", bufs=4))
            cpool = pa.enter_context(tc.tile_pool(name="cospool", bufs=1))
            ps_a = pa.enter_context(
                tc.tile_pool(name="ps_a", bufs=3, space="PSUM"))

            wq_sb = wpool.tile([128, NIC, DLOC], bf16, name="wq_sb")
            wk_sb = wpool.tile([128, NIC, DLOC], bf16, name="wk_sb")
            wv_sb = wpool.tile([128, NIC, DLOC], bf16, name="wv_sb")
            cos2_sb = cpool.tile([HD, SEQ], bf16, name="cos2_sb")
            sinpm_sb = cpool.tile([HD, SEQ], bf16, name="sinpm_sb")

            nc.sync.dma_start(out=wq_sb[:], in_=wqT_d.rearrange("(c p) d -> p c d", p=128))
            nc.sync.dma_start(out=wk_sb[:], in_=wkT_d.rearrange("(c p) d -> p c d", p=128))
            nc.sync.dma_start(out=wv_sb[:], in_=wvT_d.rearrange("(c p) d -> p c d", p=128))
            nc.sync.dma_start(out=cos2_sb[:], in_=cos2_d[:, :])
            nc.sync.dma_start(out=sinpm_sb[:], in_=sinpm_d[:, :])

            xT_r = xT_d.rearrange("(c p) s -> p c s", p=128)

            for qc in range(NQC):
                x_sb = xpool.tile([128, NIC, QC], bf16, name="x_sb")
                nc.sync.dma_start(out=x_sb[:], in_=xT_r[:, :, ts(qc, QC)])

                # Q and K passes (transposed layout), with RoPE on eviction
                for w_sb, dst in ((wq_sb, qt_sb), (wk_sb, kt_sb)):
                    for h in range(HPC):
                        acc = ps_a.tile([128, QC], mybir.dt.float32, name="acc",
                                        tag="ps_a")
                        for ic in range(NIC):
                            nc.tensor.matmul(
                                acc[:],
                                w_sb[:, ic, ts(h, HD)],
                                x_sb[:, ic, :],
                                start=(ic == 0), stop=(ic == NIC - 1))
                        # RoPE: out = acc*cos2 + swap_halves(acc)*sinpm
                        sw = rpool.tile([128, QC], mybir.dt.float32, name="sw")
                        m1 = rpool.tile([128, QC], mybir.dt.float32, name="m1")
                        nc.vector.tensor_copy(sw[0:64, :], acc[64:128, :])
                        nc.vector.tensor_copy(sw[64:128, :], acc[0:64, :])
                        nc.vector.tensor_mul(m1[:], acc[:], cos2_sb[:, ts(qc, QC)])
                        nc.vector.tensor_mul(sw[:], sw[:], sinpm_sb[:, ts(qc, QC)])
                        nc.vector.tensor_add(dst[:, h, ts(qc, QC)], m1[:], sw[:])

                # V pass (natural layout)
                for sl in range(QC // 128):
                    s = qc * (QC // 128) + sl
                    acc = ps_a.tile([128, DLOC], mybir.dt.float32, name="acc",
                                    tag="ps_a")
                    for ic in range(NIC):
                        nc.tensor.matmul(
                            acc[:],
                            x_sb[:, ic, ts(sl, 128)],
                            wv_sb[:, ic, :],
                            start=(ic == 0), stop=(ic == NIC - 1))
                    nc.vector.tensor_copy(v_sb[:, s, :], acc[:])

        # ---------------- Phase B: attention ----------------
        ag_outs = []
        with ExitStack() as pb:
            ptpool = pb.enter_context(tc.tile_pool(name="ptpool", bufs=3))
            rbpool = pb.enter_context(tc.tile_pool(name="rbpool", bufs=2))
            rpool2 = pb.enter_context(tc.tile_pool(name="rpool2", bufs=2))
            ps_st = pb.enter_context(
                tc.tile_pool(name="ps_st", bufs=2, space="PSUM"))
            ps_pv = pb.enter_context(
                tc.tile_pool(name="ps_pv", bufs=2, space="PSUM"))
            ps_on = pb.enter_context(
                tc.tile_pool(name="ps_on", bufs=1, space="PSUM"))

            for h in range(HPC):
                for qc in range(NQC):
                    kts = block_plan[qc]
                    nkt = len(kts)
                    pv = ps_pv.tile([128, QC], mybir.dt.float32, name="pv")
                    csum = ps_on.tile([1, QC], mybir.dt.float32, name="csum")
                    # pair up k-tiles so exp runs on [128, 2*QC] blocks
                    for pi in range(0, nkt, 2):
                        pair = kts[pi:pi + 2]
                        st = ps_st.tile([128, 2, QC], mybir.dt.float32, name="st")
                        for j, (kt, _) in enumerate(pair):
                            nc.tensor.matmul(
                                st[:, j, :],
                                kt_sb[:, h, ts(kt, KT)],
                                qt_sb[:, h, ts(qc, QC)],
                                start=True, stop=True)
                        pt = ptpool.tile([128, 2, QC], bf16, name="pt")
                        nc.scalar.activation(
                            pt[:, 0:len(pair), :], st[:, 0:len(pair), :],
                            mybir.ActivationFunctionType.Exp)
                        for j, (kt, pidx) in enumerate(pair):
                            if pidx is not None:
                                nc.vector.tensor_mul(
                                    pt[:, j, :], pt[:, j, :],
                                    pat_sb[:, pidx, :])
                            i = pi + j
                            nc.tensor.matmul(
                                csum[:], ones_sb[:],
                                pt[:, j, :],
                                start=(i == 0), stop=(i == nkt - 1))
                            nc.tensor.matmul(
                                pv[:], v_sb[:, kt, ts(h, HD)],
                                pt[:, j, :],
                                start=(i == 0), stop=(i == nkt - 1))
                    r = rpool2.tile([1, QC], mybir.dt.float32, name="r")
                    rb = rbpool.tile([128, QC], mybir.dt.float32, name="rb")
                    nc.vector.reciprocal(r[:], csum[:])
                    nc.gpsimd.partition_broadcast(rb[:], r[:])
                    nc.vector.tensor_mul(at_sb[:, h, ts(qc, QC)], pv[:], rb[:])

                # per-head AllGather across the batch group
                ag_in = dram.tile([128, SEQ], bf16, name="ag_in")
                ag_out = dram.tile([GSIZE * 128, SEQ], bf16, name="ag_out")
                nc.sync.dma_start(out=ag_in[:], in_=at_sb[:, h, :])
                nc.gpsimd.collective_compute(
                    "AllGather", mybir.AluOpType.bypass,
                    replica_groups=groups,
                    ins=[ag_in[:].opt()],
                    outs=[ag_out[:].opt()])
                ag_outs.append(ag_out)

        # ---------------- Phase C: output projection ----------------
        with ExitStack() as pc:
            wopool = pc.enter_context(tc.tile_pool(name="wopool", bufs=1))
            ltpool = pc.enter_context(tc.tile_pool(name="ltpool", bufs=4))
            opool = pc.enter_context(tc.tile_pool(name="opool", bufs=3))
            ps_c = pc.enter_context(
                tc.tile_pool(name="ps_c", bufs=2, space="PSUM"))

            wo_sb = wopool.tile([128, NIC, DLOC], bf16, name="wo_sb")
            nc.sync.dma_start(out=wo_sb[:], in_=woT_d.rearrange("(c p) d -> p c d", p=128))

            for s in range(NKT):
                acc = ps_c.tile([128, DLOC], mybir.dt.float32, name="acc_c")
                for c in range(NIC):
                    hh, rr = c % GSIZE, c // GSIZE
                    lt = ltpool.tile([128, 128], bf16, name="lt")
                    nc.sync.dma_start(
                        out=lt[:],
                        in_=ag_outs[hh][ts(rr, 128), ts(s, 128)])
                    nc.tensor.matmul(acc[:], lt[:], wo_sb[:, c, :],
                                     start=(c == 0), stop=(c == NIC - 1))
                ot = opool.tile([128, DLOC], mybir.dt.float32, name="ot")
                nc.vector.tensor_copy(ot[:], acc[:])
                nc.sync.dma_start(out=out_d[ts(s, 128), :], in_=ot[:])

    nc.compile()
    return nc


_CACHE = {}


def _get_compiled(block_plan_key, n_pat):
    key = (block_plan_key, n_pat)
    if key not in _CACHE:
        _CACHE[key] = _build_and_compile(block_plan_key, n_pat)
    return _CACHE[key]


def _plan_from_mask(mask):
    """Derive per-q-chunk k-tile lists + dedup'd 0/1 patterns from the mask."""
    keep = mask > -1e20
    if not np.all(mask[keep] == 0.0):
        raise NotImplementedError("only 0/-inf style masks supported")
    pats = []
    pat_index = {}
    plan = []
    for qc in range(NQC):
        qs = slice(qc * QC, (qc + 1) * QC)
        row = []
        for kt in range(NKT):
            ks = slice(kt * KT, (kt + 1) * KT)
            blk = keep[qs, ks]            # [QC, KT]
            if not blk.any():
                continue
            if blk.all():
                row.append((kt, -1))
            else:
                p = np.ascontiguousarray(blk.T).astype(np.float32)  # [KT, QC]
                kb = p.tobytes()
                if kb not in pat_index:
                    pat_index[kb] = len(pats)
                    pats.append(p)
                row.append((kt, pat_index[kb]))
        plan.append(tuple(row))
    return tuple(plan), pats


def _head_perm():
    """Row permutation per head: even dims first, then odd."""
    perm = []
    for h in range(NH):
        base = h * HD
        perm.extend(base + np.arange(0, HD, 2))
        perm.extend(base + np.arange(1, HD, 2))
    return np.array(perm)


def kernel(x, wq, wk, wv, wo, freqs_cos, freqs_sin, mask):
    x = np.asarray(x, dtype=np.float32)
    wq = np.asarray(wq, dtype=np.float32)
    wk = np.asarray(wk, dtype=np.float32)
    wv = np.asarray(wv, dtype=np.float32)
    wo = np.asarray(wo, dtype=np.float32)
    freqs_cos = np.asarray(freqs_cos, dtype=np.float32)
    freqs_sin = np.asarray(freqs_sin, dtype=np.float32)
    mask = np.asarray(mask, dtype=np.float32)

    plan, pats = _plan_from_mask(mask)
    n_pat = len(pats)
    nc = _get_compiled(plan, n_pat)

    perm = _head_perm()
    wq_p = (wq / math.sqrt(HD))[perm]
    wk_p = wk[perm]

    cosT = np.ascontiguousarray(freqs_cos.T)        # [64, SEQ]
    sinT = np.ascontiguousarray(freqs_sin.T)
    cos2 = np.concatenate([cosT, cosT], axis=0).astype(BF16)   # [128, SEQ]
    sinpm = np.concatenate([-sinT, sinT], axis=0).astype(BF16)

    if n_pat:
        pat_np = np.stack(pats).astype(BF16)        # [n_pat, KT, QC]
    else:
        pat_np = np.zeros((1, KT, QC), dtype=BF16)

    xT = [np.ascontiguousarray(x[b].T).astype(BF16) for b in range(BSZ)]

    in_maps = []
    for c in range(NCORES):
        b, g = c // GSIZE, c % GSIZE
        rows = slice(g * DLOC, (g + 1) * DLOC)
        in_maps.append({
            "xT": xT[b],
            "wqT": np.ascontiguousarray(wq_p[rows].T).astype(BF16),
            "wkT": np.ascontiguousarray(wk_p[rows].T).astype(BF16),
            "wvT": np.ascontiguousarray(wv[rows].T).astype(BF16),
            "woT": np.ascontiguousarray(wo[rows].T).astype(BF16),
            "cos2": cos2,
            "sinpm": sinpm,
            "pat": pat_np,
        })

    from concourse.bass_utils import run_bass_kernel_spmd
    res = run_bass_kernel_spmd(nc, in_maps, core_ids=list(range(NCORES)))
    outs = res.results

    full = np.empty((BSZ, SEQ, DIM), dtype=np.float32)
    for c in range(NCORES):
        b, g = c // GSIZE, c % GSIZE
        full[b][:, g * DLOC:(g + 1) * DLOC] = outs[c]["out"]
    return full


# revision 5
# speedup vs baseline: 1.5160x; 1.5160x over previous
"""Distributed multi-head causal attention with RoPE on 8 TRN2 NeuronCores.

Sharding: batch (2) x head-groups (4 heads each) -> 8 cores.
  core c: batch b = c // 4, head group g = c % 4 (global heads 4g..4g+3).

Per-core kernel (all matmuls bf16, fp32 accumulate):
  1. QKV projections in transposed layout: QT/KT[d, seq] (head dim on
     partitions), V[seq, dv] natural.  RoPE pair-interleave is folded into a
     host-side permutation of wq/wk rows (evens-first), so on-device RoPE is
     3 elementwise ops + a half-swap copy.  The 1/sqrt(hd) scale is folded
     into wq on the host.
  2. Scores computed transposed: ST[k, q] = KT_blk.T @ QT (so softmax'd
     probabilities come out in the exact layout PV needs as its moving
     operand).  exp on ACT (no max subtraction -- scores are O(1) for this
     problem), fully-masked blocks skipped structurally, partial blocks
     masked by a 0/1 pattern multiply.  Column sums via an all-ones [128,128]
     matmul (gives the sum replicated across partitions, so the reciprocal
     runs full-width); normalization is applied to the PV output.
  3. Per-head AllGather (bf16) of normalized attnT across the 4 cores of the
     batch group, overlapped with attention of later heads; gathered heads
     are staged back into SBUF as they arrive.
  4. Output projection column-sharded: each core computes its 512 output
     columns from the gathered attnT; host concatenates.
"""

import functools
import math

import numpy as np
import ml_dtypes

BSZ, SEQ, DIM, NH, HD = 2, 2048, 2048, 16, 128
NCORES = 8
GSIZE = 4            # cores per batch group
HPC = NH // GSIZE    # heads per core = 4
DLOC = HPC * HD      # local head dims = 512
QC = 512             # q-chunk (matmul moving free dim)
NQC = SEQ // QC      # 4
KT = 128             # k-tile
NKT = SEQ // KT      # 16
IC = 128             # contraction tile
NIC = DIM // IC      # 16
BF16 = ml_dtypes.bfloat16
NEG_BIG = -30000.0


def _build_and_compile(block_plan_key, n_pat):
    """Build + compile the SPMD bass graph.  block_plan_key is a tuple over
    q-chunks of tuples of (kt, pat_idx or -1)."""
    import concourse.bass as bass
    import concourse.tile as tile
    from concourse import bacc, mybir
    from contextlib import ExitStack

    f32 = mybir.dt.float32
    bf16 = mybir.dt.bfloat16
    ts = bass.ts

    block_plan = [[(kt, (None if p < 0 else p)) for kt, p in qcp]
                  for qcp in block_plan_key]

    nc = bacc.Bacc("TRN2", target_bir_lowering=False, debug=False,
                   num_devices=NCORES)

    xT_d = nc.dram_tensor("xT", [DIM, SEQ], bf16, kind="ExternalInput").ap()
    wqT_d = nc.dram_tensor("wqT", [DIM, DLOC], bf16, kind="ExternalInput").ap()
    wkT_d = nc.dram_tensor("wkT", [DIM, DLOC], bf16, kind="ExternalInput").ap()
    wvT_d = nc.dram_tensor("wvT", [DIM, DLOC], bf16, kind="ExternalInput").ap()
    woT_d = nc.dram_tensor("woT", [DIM, DLOC], bf16, kind="ExternalInput").ap()
    cos2_d = nc.dram_tensor("cos2", [HD, SEQ], bf16, kind="ExternalInput").ap()
    sinpm_d = nc.dram_tensor("sinpm", [HD, SEQ], bf16, kind="ExternalInput").ap()
    pat_d = nc.dram_tensor("pat", [max(n_pat, 1), KT, QC], bf16,
                           kind="ExternalInput").ap()
    out_d = nc.dram_tensor("out", [SEQ, DLOC], f32, kind="ExternalOutput").ap()

    groups = [[0, 1, 2, 3], [4, 5, 6, 7]]

    with tile.TileContext(nc) as tc, ExitStack() as top:
        persist = top.enter_context(tc.tile_pool(name="persist", bufs=1))
        dram = top.enter_context(
            tc.tile_pool(name="dram", bufs=2 * HPC, space="DRAM"))

        qt_sb = persist.tile([128, HPC, SEQ], bf16, name="qt_sb")
        kt_sb = persist.tile([128, HPC, SEQ], bf16, name="kt_sb")
        v_sb = persist.tile([128, NKT, DLOC], bf16, name="v_sb")
        at_sb = persist.tile([128, HPC, SEQ], bf16, name="at_sb")
        ones_sb = persist.tile([128, 128], bf16, name="ones_sb")
        pat_sb = persist.tile([128, max(n_pat, 1), QC], bf16, name="pat_sb")

        nc.vector.memset(ones_sb[:], 1.0)

        # ---------------- Phase A: QKV projections + RoPE ----------------
        with ExitStack() as pa:
            wpool = pa.enter_context(tc.tile_pool(name="wpool", bufs=1))
            xpool = pa.enter_context(tc.tile_pool(name="xpool", bufs=2 * NIC))
            rpool = pa.enter_context(tc.tile_pool(name="rope", bufs=4))
            cpool = pa.enter_context(tc.tile_pool(name="cospool", bufs=1))
            ps_a = pa.enter_context(
                tc.tile_pool(name="ps_a", bufs=3, space="PSUM"))

            # per-chunk weight tiles so the first matmuls only wait on their
            # own 128KB DMA
            wq_sb = [wpool.tile([128, DLOC], bf16, name=f"wq_sb{i}")
                     for i in range(NIC)]
            wk_sb = [wpool.tile([128, DLOC], bf16, name=f"wk_sb{i}")
                     for i in range(NIC)]
            wv_sb = [wpool.tile([128, DLOC], bf16, name=f"wv_sb{i}")
                     for i in range(NIC)]
            cos2_sb = cpool.tile([HD, SEQ], bf16, name="cos2_sb")
            sinpm_sb = cpool.tile([HD, SEQ], bf16, name="sinpm_sb")

            xT_r = xT_d.rearrange("(c p) s -> p c s", p=128)
            x_sb = {}

            def load_x(qc):
                tiles = []
                for ic in range(NIC):
                    t = xpool.tile([128, QC], bf16, name="x_sb", tag="x_sb")
                    nc.sync.dma_start(out=t[:], in_=xT_r[:, ic, ts(qc, QC)])
                    tiles.append(t)
                x_sb[qc] = tiles

            # DMA order: interleave wq chunks with x(qc=0) chunks, then the
            # rest -- lets the first accumulation start after ~2 small DMAs.
            load_x(0)
            for i in range(NIC):
                nc.sync.dma_start(out=wq_sb[i][:],
                                  in_=wqT_d[ts(i, 128), :])
            for i in range(NIC):
                nc.sync.dma_start(out=wk_sb[i][:],
                                  in_=wkT_d[ts(i, 128), :])
            for i in range(NIC):
                nc.sync.dma_start(out=wv_sb[i][:],
                                  in_=wvT_d[ts(i, 128), :])
            nc.scalar.dma_start(out=cos2_sb[:], in_=cos2_d[:, :])
            nc.scalar.dma_start(out=sinpm_sb[:], in_=sinpm_d[:, :])
            nc.scalar.dma_start(out=pat_sb[:],
                                in_=pat_d.rearrange("n p q -> p n q"))

            for qc in range(NQC):
                if qc + 1 < NQC:
                    load_x(qc + 1)
                xs = x_sb.pop(qc)

                # Q and K passes (transposed layout), with RoPE on eviction
                for w_sb, dst in ((wq_sb, qt_sb), (wk_sb, kt_sb)):
                    for h in range(HPC):
                        acc = ps_a.tile([128, QC], f32, name="acc",
                                        tag="ps_a")
                        for ic in range(NIC):
                            nc.tensor.matmul(
                                acc[:],
                                w_sb[ic][:, ts(h, HD)],
                                xs[ic][:],
                                start=(ic == 0), stop=(ic == NIC - 1))
                        # RoPE: out = acc*cos2 + swap_halves(acc)*sinpm
                        sw = rpool.tile([128, QC], f32, name="sw")
                        m1 = rpool.tile([128, QC], f32, name="m1")
                        nc.vector.tensor_copy(sw[0:64, :], acc[64:128, :])
                        nc.vector.tensor_copy(sw[64:128, :], acc[0:64, :])
                        nc.vector.tensor_mul(m1[:], acc[:], cos2_sb[:, ts(qc, QC)])
                        nc.vector.tensor_mul(sw[:], sw[:], sinpm_sb[:, ts(qc, QC)])
                        nc.vector.tensor_add(dst[:, h, ts(qc, QC)], m1[:], sw[:])

                # V pass (natural layout)
                for sl in range(QC // 128):
                    s = qc * (QC // 128) + sl
                    acc = ps_a.tile([128, DLOC], f32, name="acc",
                                    tag="ps_a")
                    for ic in range(NIC):
                        nc.tensor.matmul(
                            acc[:],
                            xs[ic][:, ts(sl, 128)],
                            wv_sb[ic][:],
                            start=(ic == 0), stop=(ic == NIC - 1))
                    nc.vector.tensor_copy(v_sb[:, s, :], acc[:])

        # ---------------- Phase B: attention ----------------
        # gathered attnT staged back to SBUF: one tile per local head index.
        # Allocated after phase A's pools closed so it reuses their space.
        aopool = top.enter_context(tc.tile_pool(name="aopool", bufs=1))
        ao_sb = [aopool.tile([128, GSIZE, SEQ], bf16, name=f"ao_sb{h}")
                 for h in range(HPC)]
        with ExitStack() as pb:
            ptpool = pb.enter_context(tc.tile_pool(name="ptpool", bufs=4))
            rbpool = pb.enter_context(tc.tile_pool(name="rbpool", bufs=2))
            ps_st = pb.enter_context(
                tc.tile_pool(name="ps_st", bufs=2, space="PSUM"))
            ps_pv = pb.enter_context(
                tc.tile_pool(name="ps_pv", bufs=2, space="PSUM"))
            ps_on = pb.enter_context(
                tc.tile_pool(name="ps_on", bufs=2, space="PSUM"))

            for h in range(HPC):
                for qc in range(NQC):
                    kts = block_plan[qc]
                    nkt = len(kts)
                    pv = ps_pv.tile([128, QC], f32, name="pv")
                    csum = ps_on.tile([128, QC], f32, name="csum")
                    # pair up k-tiles so exp runs on [128, 2*QC] blocks
                    for pi in range(0, nkt, 2):
                        pair = kts[pi:pi + 2]
                        st = ps_st.tile([128, 2, QC], f32, name="st")
                        for j, (kt, _) in enumerate(pair):
                            nc.tensor.matmul(
                                st[:, j, :],
                                kt_sb[:, h, ts(kt, KT)],
                                qt_sb[:, h, ts(qc, QC)],
                                start=True, stop=True)
                        pt = ptpool.tile([128, 2, QC], bf16, name="pt")
                        nc.scalar.activation(
                            pt[:, 0:len(pair), :], st[:, 0:len(pair), :],
                            mybir.ActivationFunctionType.Exp)
                        for j, (kt, pidx) in enumerate(pair):
                            if pidx is not None:
                                nc.vector.tensor_mul(
                                    pt[:, j, :], pt[:, j, :],
                                    pat_sb[:, pidx, :])
                            i = pi + j
                            nc.tensor.matmul(
                                csum[:], ones_sb[:],
                                pt[:, j, :],
                                start=(i == 0), stop=(i == nkt - 1))
                            nc.tensor.matmul(
                                pv[:], v_sb[:, kt, ts(h, HD)],
                                pt[:, j, :],
                                start=(i == 0), stop=(i == nkt - 1))
                    # csum rows are all identical (ones matmul), so the
                    # reciprocal runs full-width straight out of PSUM.
                    rb = rbpool.tile([128, QC], f32, name="rb")
                    nc.vector.reciprocal(rb[:], csum[:])
                    nc.vector.tensor_mul(at_sb[:, h, ts(qc, QC)], pv[:], rb[:])

                # per-head AllGather across the batch group
                ag_in = dram.tile([128, SEQ], bf16, name="ag_in")
                ag_out = dram.tile([GSIZE * 128, SEQ], bf16, name="ag_out")
                nc.sync.dma_start(out=ag_in[:], in_=at_sb[:, h, :])
                nc.gpsimd.collective_compute(
                    "AllGather", mybir.AluOpType.bypass,
                    replica_groups=groups,
                    ins=[ag_in[:].opt()],
                    outs=[ag_out[:].opt()])
                # stage the gathered heads back into SBUF (one big DMA)
                nc.gpsimd.dma_start(
                    out=ao_sb[h][:],
                    in_=ag_out.rearrange("(r p) s -> p r s", p=128))

        # ---------------- Phase C: output projection ----------------
        with ExitStack() as pc:
            wopool = pc.enter_context(tc.tile_pool(name="wopool", bufs=1))
            opool = pc.enter_context(tc.tile_pool(name="opool", bufs=3))
            ps_c = pc.enter_context(
                tc.tile_pool(name="ps_c", bufs=2, space="PSUM"))

            wo_sb = wopool.tile([128, NIC, DLOC], bf16, name="wo_sb")
            nc.sync.dma_start(out=wo_sb[:],
                              in_=woT_d.rearrange("(c p) d -> p c d", p=128))

            for s in range(NKT):
                acc = ps_c.tile([128, DLOC], f32, name="acc_c")
                first, last = (0, 0), (HPC - 1, GSIZE - 1)
                for h in range(HPC):          # later heads last: AG overlap
                    for r in range(GSIZE):
                        gh = GSIZE * r + h
                        nc.tensor.matmul(
                            acc[:], ao_sb[h][:, r, ts(s, 128)],
                            wo_sb[:, gh, :],
                            start=((h, r) == first), stop=((h, r) == last))
                ot = opool.tile([128, DLOC], f32, name="ot")
                nc.vector.tensor_copy(ot[:], acc[:])
                nc.sync.dma_start(out=out_d[ts(s, 128), :], in_=ot[:])

    nc.compile()
    return nc


_CACHE = {}


def _get_compiled(block_plan_key, n_pat):
    key = (block_plan_key, n_pat)
    if key not in _CACHE:
        _CACHE[key] = _build_and_compile(block_plan_key, n_pat)
    return _CACHE[key]


def _plan_from_mask(mask):
    """Derive per-q-chunk k-tile lists + dedup'd 0/1 patterns from the mask."""
    keep = mask > -1e20
    if not np.all(mask[keep] == 0.0):
        raise NotImplementedError("only 0/-inf style masks supported")
    pats = []
    pat_index = {}
    plan = []
    for qc in range(NQC):
        qs = slice(qc * QC, (qc + 1) * QC)
        row = []
        for kt in range(NKT):
            ks = slice(kt * KT, (kt + 1) * KT)
            blk = keep[qs, ks]            # [QC, KT]
            if not blk.any():
                continue
            if blk.all():
                row.append((kt, -1))
            else:
                p = np.ascontiguousarray(blk.T).astype(np.float32)  # [KT, QC]
                kb = p.tobytes()
                if kb not in pat_index:
                    pat_index[kb] = len(pats)
                    pats.append(p)
                row.append((kt, pat_index[kb]))
        plan.append(tuple(row))
    return tuple(plan), pats


def _head_perm():
    """Row permutation per head: even dims first, then odd."""
    perm = []
    for h in range(NH):
        base = h * HD
        perm.extend(base + np.arange(0, HD, 2))
        perm.extend(base + np.arange(1, HD, 2))
    return np.array(perm)


def _prep_in_maps(x, wq, wk, wv, wo, freqs_cos, freqs_sin, pats, n_pat):
    perm = _head_perm()
    wq_p = (wq / math.sqrt(HD))[perm]
    wk_p = wk[perm]

    cosT = np.ascontiguousarray(freqs_cos.T)        # [64, SEQ]
    sinT = np.ascontiguousarray(freqs_sin.T)
    cos2 = np.concatenate([cosT, cosT], axis=0).astype(BF16)   # [128, SEQ]
    sinpm = np.concatenate([-sinT, sinT], axis=0).astype(BF16)

    if n_pat:
        pat_np = np.stack(pats).astype(BF16)        # [n_pat, KT, QC]
    else:
        pat_np = np.zeros((1, KT, QC), dtype=BF16)

    xT = [np.ascontiguousarray(x[b].T).astype(BF16) for b in range(BSZ)]

    in_maps = []
    for c in range(NCORES):
        b, g = c // GSIZE, c % GSIZE
        rows = slice(g * DLOC, (g + 1) * DLOC)
        in_maps.append({
            "xT": xT[b],
            "wqT": np.ascontiguousarray(wq_p[rows].T).astype(BF16),
            "wkT": np.ascontiguousarray(wk_p[rows].T).astype(BF16),
            "wvT": np.ascontiguousarray(wv[rows].T).astype(BF16),
            "woT": np.ascontiguousarray(wo[rows].T).astype(BF16),
            "cos2": cos2,
            "sinpm": sinpm,
            "pat": pat_np,
        })
    return in_maps


def kernel(x, wq, wk, wv, wo, freqs_cos, freqs_sin, mask):
    x = np.asarray(x, dtype=np.float32)
    wq = np.asarray(wq, dtype=np.float32)
    wk = np.asarray(wk, dtype=np.float32)
    wv = np.asarray(wv, dtype=np.float32)
    wo = np.asarray(wo, dtype=np.float32)
    freqs_cos = np.asarray(freqs_cos, dtype=np.float32)
    freqs_sin = np.asarray(freqs_sin, dtype=np.float32)
    mask = np.asarray(mask, dtype=np.float32)

    plan, pats = _plan_from_mask(mask)
    n_pat = len(pats)
    nc = _get_compiled(plan, n_pat)

    in_maps = _prep_in_maps(x, wq, wk, wv, wo, freqs_cos, freqs_sin,
                            pats, n_pat)

    from concourse.bass_utils import run_bass_kernel_spmd
    res = run_bass_kernel_spmd(nc, in_maps, core_ids=list(range(NCORES)))
    outs = res.results

    full = np.empty((BSZ, SEQ, DIM), dtype=np.float32)
    for c in range(NCORES):
        b, g = c // GSIZE, c % GSIZE
        full[b][:, g * DLOC:(g + 1) * DLOC] = outs[c]["out"]
    return full


# revision 10
# speedup vs baseline: 1.6606x; 1.0954x over previous
"""Distributed multi-head causal attention with RoPE on 8 TRN2 NeuronCores.

Sharding: batch (2) x head-groups (4 heads each) -> 8 cores.
  core c: batch b = c // 4, head group g = c % 4 (global heads 4g..4g+3).

Per-core kernel (all matmuls bf16, fp32 accumulate):
  1. QKV projections in transposed layout: QT/KT[d, seq] (head dim on
     partitions), V[seq, dv] natural.  RoPE pair-interleave is folded into a
     host-side permutation of wq/wk rows (evens-first), so on-device RoPE is
     3 elementwise ops + a half-swap copy.  The 1/sqrt(hd) scale is folded
     into wq on the host.
  2. Scores computed transposed: ST[k, q] = KT_blk.T @ QT (so softmax'd
     probabilities come out in the exact layout PV needs as its moving
     operand).  exp on ACT (no max subtraction -- scores are O(1) for this
     problem), fully-masked blocks skipped structurally, partial blocks
     masked by a 0/1 pattern multiply.  Column sums via an all-ones [128,128]
     matmul (gives the sum replicated across partitions, so the reciprocal
     runs full-width); normalization is applied to the PV output.
  3. Per-head AllGather (bf16) of normalized attnT across the 4 cores of the
     batch group, overlapped with attention of later heads; gathered heads
     are staged back into SBUF as they arrive.
  4. Output projection column-sharded: each core computes its 512 output
     columns from the gathered attnT; host concatenates.
"""

import functools
import math

import numpy as np
import ml_dtypes

BSZ, SEQ, DIM, NH, HD = 2, 2048, 2048, 16, 128
NCORES = 8
GSIZE = 4            # cores per batch group
HPC = NH // GSIZE    # heads per core = 4
DLOC = HPC * HD      # local head dims = 512
QC = 512             # q-chunk (matmul moving free dim)
NQC = SEQ // QC      # 4
KT = 128             # k-tile
NKT = SEQ // KT      # 16
IC = 128             # contraction tile
NIC = DIM // IC      # 16
BF16 = ml_dtypes.bfloat16
NEG_BIG = -30000.0


def _build_and_compile(block_plan_key, n_pat):
    """Build + compile the SPMD bass graph.  block_plan_key is a tuple over
    q-chunks of tuples of (kt, pat_idx or -1)."""
    import concourse.bass as bass
    import concourse.tile as tile
    from concourse import bacc, mybir
    from contextlib import ExitStack

    f32 = mybir.dt.float32
    bf16 = mybir.dt.bfloat16
    ts = bass.ts

    block_plan = [[(kt, (None if p < 0 else p)) for kt, p in qcp]
                  for qcp in block_plan_key]

    nc = bacc.Bacc("TRN2", target_bir_lowering=False, debug=False,
                   num_devices=NCORES)

    xT_d = nc.dram_tensor("xT", [DIM, SEQ], bf16, kind="ExternalInput").ap()
    wqT_d = nc.dram_tensor("wqT", [DIM, DLOC], bf16, kind="ExternalInput").ap()
    wkT_d = nc.dram_tensor("wkT", [DIM, DLOC], bf16, kind="ExternalInput").ap()
    wvT_d = nc.dram_tensor("wvT", [DIM, DLOC], bf16, kind="ExternalInput").ap()
    woT_d = nc.dram_tensor("woT", [DIM, DLOC], bf16, kind="ExternalInput").ap()
    cos2_d = nc.dram_tensor("cos2", [HD, SEQ], bf16, kind="ExternalInput").ap()
    sinpm_d = nc.dram_tensor("sinpm", [HD, SEQ], bf16, kind="ExternalInput").ap()
    pat_d = nc.dram_tensor("pat", [max(n_pat, 1), KT, QC], bf16,
                           kind="ExternalInput").ap()
    out_d = nc.dram_tensor("out", [SEQ, DLOC], f32, kind="ExternalOutput").ap()

    groups = [[0, 1, 2, 3], [4, 5, 6, 7]]

    with tile.TileContext(nc) as tc, ExitStack() as top:
        persist = top.enter_context(tc.tile_pool(name="persist", bufs=1))
        dram = top.enter_context(
            tc.tile_pool(name="dram", bufs=2 * HPC, space="DRAM"))

        qt_sb = persist.tile([128, HPC, SEQ], bf16, name="qt_sb")
        kt_sb = persist.tile([128, HPC, SEQ], bf16, name="kt_sb")
        v_sb = persist.tile([128, NKT, DLOC], bf16, name="v_sb")
        at_sb = persist.tile([128, HPC, SEQ], bf16, name="at_sb")
        ones_sb = persist.tile([128, 128], bf16, name="ones_sb")
        pat_sb = persist.tile([128, max(n_pat, 1), QC], bf16, name="pat_sb")

        nc.vector.memset(ones_sb[:], 1.0)

        # ---------------- Phase A: QKV projections + RoPE ----------------
        with ExitStack() as pa:
            wpool = pa.enter_context(tc.tile_pool(name="wpool", bufs=1))
            xpool = pa.enter_context(tc.tile_pool(name="xpool", bufs=2 * NIC))
            rpool = pa.enter_context(tc.tile_pool(name="rope", bufs=4))
            cpool = pa.enter_context(tc.tile_pool(name="cospool", bufs=1))
            ps_a = pa.enter_context(
                tc.tile_pool(name="ps_a", bufs=3, space="PSUM"))

            # per-chunk weight tiles so the first matmuls only wait on their
            # own 128KB DMA
            wq_sb = [wpool.tile([128, DLOC], bf16, name=f"wq_sb{i}")
                     for i in range(NIC)]
            wk_sb = [wpool.tile([128, DLOC], bf16, name=f"wk_sb{i}")
                     for i in range(NIC)]
            wv_sb = [wpool.tile([128, DLOC], bf16, name=f"wv_sb{i}")
                     for i in range(NIC)]
            cos2_sb = cpool.tile([HD, SEQ], bf16, name="cos2_sb")
            sinpm_sb = cpool.tile([HD, SEQ], bf16, name="sinpm_sb")

            xT_r = xT_d.rearrange("(c p) s -> p c s", p=128)
            x_sb = {}

            def load_x(qc):
                tiles = []
                for ic in range(NIC):
                    t = xpool.tile([128, QC], bf16, name="x_sb", tag="x_sb")
                    nc.sync.dma_start(out=t[:], in_=xT_r[:, ic, ts(qc, QC)])
                    tiles.append(t)
                x_sb[qc] = tiles

            # Startup DMAs spread across engine queues so the streams run in
            # parallel: x -> sync, wq -> vector, wk -> scalar, wv -> gpsimd.
            load_x(0)
            for i in range(NIC):
                nc.scalar.dma_start(out=wq_sb[i][:],
                                    in_=wqT_d[ts(i, 128), :])
            for i in range(NIC):
                nc.gpsimd.dma_start(out=wk_sb[i][:],
                                    in_=wkT_d[ts(i, 128), :])
            for i in range(NIC):
                nc.sync.dma_start(out=wv_sb[i][:],
                                  in_=wvT_d[ts(i, 128), :])
            nc.scalar.dma_start(out=cos2_sb[:], in_=cos2_d[:, :])
            nc.scalar.dma_start(out=sinpm_sb[:], in_=sinpm_d[:, :])
            nc.scalar.dma_start(out=pat_sb[:],
                                in_=pat_d.rearrange("n p q -> p n q"))

            for qc in range(NQC):
                if qc + 1 < NQC:
                    load_x(qc + 1)
                xs = x_sb.pop(qc)

                # Q and K passes (transposed layout), with RoPE on eviction
                for w_sb, dst in ((wq_sb, qt_sb), (wk_sb, kt_sb)):
                    for h in range(HPC):
                        acc = ps_a.tile([128, QC], f32, name="acc",
                                        tag="ps_a")
                        for ic in range(NIC):
                            nc.tensor.matmul(
                                acc[:],
                                w_sb[ic][:, ts(h, HD)],
                                xs[ic][:],
                                start=(ic == 0), stop=(ic == NIC - 1))
                        # RoPE: out = acc*cos2 + swap_halves(acc)*sinpm
                        sw = rpool.tile([128, QC], f32, name="sw")
                        m1 = rpool.tile([128, QC], f32, name="m1")
                        nc.vector.tensor_copy(sw[0:64, :], acc[64:128, :])
                        nc.vector.tensor_copy(sw[64:128, :], acc[0:64, :])
                        nc.vector.tensor_mul(m1[:], acc[:], cos2_sb[:, ts(qc, QC)])
                        nc.vector.tensor_mul(sw[:], sw[:], sinpm_sb[:, ts(qc, QC)])
                        nc.vector.tensor_add(dst[:, h, ts(qc, QC)], m1[:], sw[:])

                # V pass (natural layout)
                for sl in range(QC // 128):
                    s = qc * (QC // 128) + sl
                    acc = ps_a.tile([128, DLOC], f32, name="acc",
                                    tag="ps_a")
                    for ic in range(NIC):
                        nc.tensor.matmul(
                            acc[:],
                            xs[ic][:, ts(sl, 128)],
                            wv_sb[ic][:],
                            start=(ic == 0), stop=(ic == NIC - 1))
                    nc.vector.tensor_copy(v_sb[:, s, :], acc[:])

        # ---------------- Phase B: attention ----------------
        # gathered attnT staged back to SBUF: one tile per local head index.
        # Allocated after phase A's pools closed so it reuses their space.
        aopool = top.enter_context(tc.tile_pool(name="aopool", bufs=1))
        ao_sb = [aopool.tile([128, GSIZE, SEQ], bf16, name=f"ao_sb{h}")
                 for h in range(HPC)]
        # wo preloads during attention
        wopool = top.enter_context(tc.tile_pool(name="wopool", bufs=1))
        wo_sb = wopool.tile([128, NIC, DLOC], bf16, name="wo_sb")
        nc.sync.dma_start(out=wo_sb[:],
                          in_=woT_d.rearrange("(c p) d -> p c d", p=128))
        with ExitStack() as pb:
            ptpool = pb.enter_context(tc.tile_pool(name="ptpool", bufs=6))
            rbpool = pb.enter_context(tc.tile_pool(name="rbpool", bufs=2))
            ps_st = pb.enter_context(
                tc.tile_pool(name="ps_st", bufs=2, space="PSUM"))
            ps_pv = pb.enter_context(
                tc.tile_pool(name="ps_pv", bufs=2, space="PSUM"))
            ps_on = pb.enter_context(
                tc.tile_pool(name="ps_on", bufs=2, space="PSUM"))

            for h in range(HPC):
                for qc in range(NQC):
                    kts = block_plan[qc]
                    nkt = len(kts)
                    pv = ps_pv.tile([128, QC], f32, name="pv")
                    csum = ps_on.tile([128, QC], f32, name="csum")
                    # pair up k-tiles so exp runs on [128, 2*QC] blocks
                    for pi in range(0, nkt, 2):
                        pair = kts[pi:pi + 2]
                        st = ps_st.tile([128, 2, QC], f32, name="st")
                        for j, (kt, _) in enumerate(pair):
                            nc.tensor.matmul(
                                st[:, j, :],
                                kt_sb[:, h, ts(kt, KT)],
                                qt_sb[:, h, ts(qc, QC)],
                                start=True, stop=True)
                        pt = ptpool.tile([128, 2, QC], bf16, name="pt")
                        nc.scalar.activation(
                            pt[:, 0:len(pair), :], st[:, 0:len(pair), :],
                            mybir.ActivationFunctionType.Exp)
                        for j, (kt, pidx) in enumerate(pair):
                            if pidx is not None:
                                nc.vector.tensor_mul(
                                    pt[:, j, :], pt[:, j, :],
                                    pat_sb[:, pidx, :])
                            i = pi + j
                            nc.tensor.matmul(
                                csum[:], ones_sb[:],
                                pt[:, j, :],
                                start=(i == 0), stop=(i == nkt - 1))
                            nc.tensor.matmul(
                                pv[:], v_sb[:, kt, ts(h, HD)],
                                pt[:, j, :],
                                start=(i == 0), stop=(i == nkt - 1))
                    # csum rows are all identical (ones matmul), so the
                    # reciprocal runs full-width straight out of PSUM.
                    rb = rbpool.tile([128, QC], f32, name="rb")
                    nc.vector.reciprocal_approx_fast(rb[:], csum[:])
                    nc.vector.tensor_mul(at_sb[:, h, ts(qc, QC)], pv[:], rb[:])

                # per-head AllGather across the batch group
                ag_in = dram.tile([128, SEQ], bf16, name="ag_in")
                ag_out = dram.tile([GSIZE * 128, SEQ], bf16, name="ag_out")
                nc.sync.dma_start(out=ag_in[:], in_=at_sb[:, h, :])
                nc.gpsimd.collective_compute(
                    "AllGather", mybir.AluOpType.bypass,
                    replica_groups=groups,
                    ins=[ag_in[:].opt()],
                    outs=[ag_out[:].opt()])
                # stage the gathered heads back into SBUF (one big DMA)
                nc.gpsimd.dma_start(
                    out=ao_sb[h][:],
                    in_=ag_out.rearrange("(r p) s -> p r s", p=128))

        # ---------------- Phase C: output projection ----------------
        # Two stages: heads 0..HPC-2 accumulate while the last head's
        # AllGather is still in flight; the last head's contribution is added
        # on top afterwards, so the AG tail hides behind ~192 matmuls.
        with ExitStack() as pc:
            popool = pc.enter_context(tc.tile_pool(name="popool", bufs=1))
            opool = pc.enter_context(tc.tile_pool(name="opool", bufs=3))
            ps_c = pc.enter_context(
                tc.tile_pool(name="ps_c", bufs=2, space="PSUM"))
            ps_c2 = pc.enter_context(
                tc.tile_pool(name="ps_c2", bufs=2, space="PSUM"))

            po_sb = popool.tile([128, NKT, DLOC], f32, name="po_sb")

            for s in range(NKT):
                acc = ps_c.tile([128, DLOC], f32, name="acc_c")
                for h in range(HPC - 1):
                    for r in range(GSIZE):
                        gh = GSIZE * r + h
                        nc.tensor.matmul(
                            acc[:], ao_sb[h][:, r, ts(s, 128)],
                            wo_sb[:, gh, :],
                            start=((h, r) == (0, 0)),
                            stop=((h, r) == (HPC - 2, GSIZE - 1)))
                nc.vector.tensor_copy(po_sb[:, s, :], acc[:])
            h = HPC - 1
            for s in range(NKT):
                acc2 = ps_c2.tile([128, DLOC], f32, name="acc2_c")
                for r in range(GSIZE):
                    nc.tensor.matmul(
                        acc2[:], ao_sb[h][:, r, ts(s, 128)],
                        wo_sb[:, GSIZE * r + h, :],
                        start=(r == 0), stop=(r == GSIZE - 1))
                ot = opool.tile([128, DLOC], f32, name="ot")
                nc.vector.tensor_add(ot[:], acc2[:], po_sb[:, s, :])
                nc.sync.dma_start(out=out_d[ts(s, 128), :], in_=ot[:])

    nc.compile()
    return nc


_CACHE = {}


def _get_compiled(block_plan_key, n_pat):
    key = (block_plan_key, n_pat)
    if key not in _CACHE:
        _CACHE[key] = _build_and_compile(block_plan_key, n_pat)
    return _CACHE[key]


def _plan_from_mask(mask):
    """Derive per-q-chunk k-tile lists + dedup'd 0/1 patterns from the mask."""
    keep = mask > -1e20
    if not np.all(mask[keep] == 0.0):
        raise NotImplementedError("only 0/-inf style masks supported")
    pats = []
    pat_index = {}
    plan = []
    for qc in range(NQC):
        qs = slice(qc * QC, (qc + 1) * QC)
        row = []
        for kt in range(NKT):
            ks = slice(kt * KT, (kt + 1) * KT)
            blk = keep[qs, ks]            # [QC, KT]
            if not blk.any():
                continue
            if blk.all():
                row.append((kt, -1))
            else:
                p = np.ascontiguousarray(blk.T).astype(np.float32)  # [KT, QC]
                kb = p.tobytes()
                if kb not in pat_index:
                    pat_index[kb] = len(pats)
                    pats.append(p)
                row.append((kt, pat_index[kb]))
        plan.append(tuple(row))
    return tuple(plan), pats


def _head_perm():
    """Row permutation per head: even dims first, then odd."""
    perm = []
    for h in range(NH):
        base = h * HD
        perm.extend(base + np.arange(0, HD, 2))
        perm.extend(base + np.arange(1, HD, 2))
    return np.array(perm)


def _prep_in_maps(x, wq, wk, wv, wo, freqs_cos, freqs_sin, pats, n_pat):
    perm = _head_perm()
    wq_p = (wq / math.sqrt(HD))[perm]
    wk_p = wk[perm]

    cosT = np.ascontiguousarray(freqs_cos.T)        # [64, SEQ]
    sinT = np.ascontiguousarray(freqs_sin.T)
    cos2 = np.concatenate([cosT, cosT], axis=0).astype(BF16)   # [128, SEQ]
    sinpm = np.concatenate([-sinT, sinT], axis=0).astype(BF16)

    if n_pat:
        pat_np = np.stack(pats).astype(BF16)        # [n_pat, KT, QC]
    else:
        pat_np = np.zeros((1, KT, QC), dtype=BF16)

    xT = [np.ascontiguousarray(x[b].T).astype(BF16) for b in range(BSZ)]

    in_maps = []
    for c in range(NCORES):
        b, g = c // GSIZE, c % GSIZE
        rows = slice(g * DLOC, (g + 1) * DLOC)
        in_maps.append({
            "xT": xT[b],
            "wqT": np.ascontiguousarray(wq_p[rows].T).astype(BF16),
            "wkT": np.ascontiguousarray(wk_p[rows].T).astype(BF16),
            "wvT": np.ascontiguousarray(wv[rows].T).astype(BF16),
            "woT": np.ascontiguousarray(wo[rows].T).astype(BF16),
            "cos2": cos2,
            "sinpm": sinpm,
            "pat": pat_np,
        })
    return in_maps


def kernel(x, wq, wk, wv, wo, freqs_cos, freqs_sin, mask):
    x = np.asarray(x, dtype=np.float32)
    wq = np.asarray(wq, dtype=np.float32)
    wk = np.asarray(wk, dtype=np.float32)
    wv = np.asarray(wv, dtype=np.float32)
    wo = np.asarray(wo, dtype=np.float32)
    freqs_cos = np.asarray(freqs_cos, dtype=np.float32)
    freqs_sin = np.asarray(freqs_sin, dtype=np.float32)
    mask = np.asarray(mask, dtype=np.float32)

    plan, pats = _plan_from_mask(mask)
    n_pat = len(pats)
    nc = _get_compiled(plan, n_pat)

    in_maps = _prep_in_maps(x, wq, wk, wv, wo, freqs_cos, freqs_sin,
                            pats, n_pat)

    from concourse.bass_utils import run_bass_kernel_spmd
    res = run_bass_kernel_spmd(nc, in_maps, core_ids=list(range(NCORES)))
    outs = res.results

    full = np.empty((BSZ, SEQ, DIM), dtype=np.float32)
    for c in range(NCORES):
        b, g = c // GSIZE, c % GSIZE
        full[b][:, g * DLOC:(g + 1) * DLOC] = outs[c]["out"]
    return full
